# revision 1
# baseline (speedup 1.0000x reference)
"""Trainium2 Bass kernel for the ContextComputer GNN message-passing module.

Computation (per batch row b):
    W1, W2 = W[:D], W[D:]
    mjm_j  = memory_j * mask_j                       # [N, D]
    a_i    = memory_i @ W1 + bias                    # [N, D]
    c_j    = mjm_j @ W2                              # [N, D]
    ctx_i  = sum_{j != i} sigmoid(a_i + c_j) * mjm_j

Sharding: pure data parallel over batch B across the 8 NeuronCores
(B=8192 -> 1024 rows per core); W/b replicated.

Per-core kernel layout: batch rows on the 128 SBUF partitions, feature
dim (D=512) on the free axis.
  - memory loads as fp32 (HWDGE), is cast-stored bf16 into a DRAM
    scratch (SWDGE cast DMA), and the d-on-partition m^T tiles for the
    matmuls come from hardware transpose-DMAs (xbar path is bf16-only)
    over row groups, alternating between both HWDGE rings.
  - u_j = mask_j * m_j via DVE tensor_scalar (per-partition scalar,
    2x mode); a'_i = m_i @ W1 + 1*bias accumulates in PSUM (bias via a
    rank-1 ones matmul); c_j = mask_j * (m_j @ W2) applies the mask as
    a per-partition scale in the ScalarE PSUM->SBUF copy.
  - Pairwise stage: one wide DVE/ACT instruction per i over all 6 j
    (diagonal included - cheaper than splitting the instruction); the
    off-diagonal 5-term j-sum is a strided paired-tree add (3 DVE ops),
    the last op writing fp32 for the output store.
"""

import numpy as np

import concourse.bass as bass
import concourse.mybir as mybir
import concourse.tile as tile
from concourse.bass_utils import run_bass_kernel_spmd

B, N, D = 8192, 6, 512
P = 128
DC = D // P  # 4 contraction chunks of 128
NCORES = 8
BLOC = B // NCORES

F32 = mybir.dt.float32
BF16 = mybir.dt.bfloat16

_ADD = mybir.AluOpType.add
_MULT = mybir.AluOpType.mult
_SIGMOID = mybir.ActivationFunctionType.Sigmoid

# per-i strided pairing of the 5 off-diagonal blocks: s = p[in0] + p[in1]
# (two blocks each, uniform strides), then s0+s1, then + p[rem]
_PAIRS = {
    0: (slice(1, 4, 2), slice(2, 5, 2), 5),
    1: (slice(0, 4, 3), slice(2, 5, 2), 5),
    2: (slice(0, 4, 3), slice(1, 5, 3), 5),
    3: (slice(0, 3, 2), slice(1, 5, 3), 5),
    4: (slice(0, 3, 2), slice(1, 4, 2), 5),
    5: (slice(0, 3, 2), slice(1, 4, 2), 4),
}

_nc_cache = {}


def _split_excess_waits(nc, max_waits=1):
    """The pinned walrus build only supports one sync-wait slot per
    instruction; hoist extra Tile-emitted waits onto standalone
    same-engine EventSemaphore instructions (NX dispatcher-level waits,
    so ordering semantics are preserved)."""
    f = nc.m.functions[0]
    for blk in f.blocks:
        new = []
        for ins in blk.instructions:
            si = getattr(ins, "sync_info", None)
            eng = getattr(ins, "engine", None)
            if si is not None and si.on_wait and len(si.on_wait) > max_waits and eng is not None:
                waits = list(si.on_wait)
                extra, keep = waits[:-max_waits], waits[-max_waits:]
                for k, w in enumerate(extra):
                    new.append(
                        mybir.InstEventSemaphore(
                            name=f"{ins.name}_xw{k}",
                            opcode="EventSemaphore",
                            engine=eng,
                            ins=[],
                            outs=[],
                            sync_info=mybir.SyncInfo(on_wait=[w], on_update=[]),
                        )
                    )
                si.on_wait = keep
            new.append(ins)
        blk.instructions[:] = new


def build(bloc=BLOC, split_waits=True):
    nbt = bloc // P
    GSZ = min(128, bloc)  # transpose group size in batch rows
    ngrp = bloc // GSZ
    bt_per_g = GSZ // P
    nc = bass.Bass(num_swdge_queues=4)
    mem = nc.declare_dram_parameter("memory", [bloc, N, D], F32, isOutput=False)
    msk = nc.declare_dram_parameter("mask", [bloc, N, 1], F32, isOutput=False)
    w_p = nc.declare_dram_parameter("W", [2 * D, D], F32, isOutput=False)
    b_p = nc.declare_dram_parameter("b", [D], F32, isOutput=False)
    out = nc.declare_dram_parameter("context", [bloc, N, D], F32, isOutput=True)
    mbf = nc.dram_tensor("mbf", [bloc, N, D], BF16)

    with tile.TileContext(nc) as tc:
        with (
            tc.tile_pool(name="const", bufs=1) as constp,
            tc.tile_pool(name="mt", bufs=2) as mtp,
            tc.tile_pool(name="work", bufs=2) as work,
            tc.tile_pool(name="pair", bufs=2) as pairp,
            tc.tile_pool(name="acc", bufs=4) as accp,
            tc.tile_pool(name="maskp", bufs=16) as maskp,
            tc.tile_pool(name="up", bufs=3) as upool,
            tc.tile_pool(name="outp", bufs=8) as outp,
            tc.tile_pool(name="psum", bufs=3, space="PSUM") as psp,
        ):
            # ---- constants: W (cast to bf16), bias, ones row ----
            wt = {}
            for h in range(2):  # 0 -> W1, 1 -> W2
                for dc in range(DC):
                    t = constp.tile([P, D], BF16, tag=f"w{h}{dc}")
                    nc.gpsimd.dma_start(
                        out=t[:], in_=w_p[h * D + dc * P : h * D + (dc + 1) * P, :]
                    )
                    wt[h, dc] = t
            bias_t = constp.tile([1, D], BF16, tag="bias")
            nc.gpsimd.dma_start(out=bias_t[:], in_=b_p[None, :])
            ones_t = constp.tile([1, P], BF16, tag="ones")
            nc.vector.memset(ones_t[:], 1.0)

            # ---- prepass, per transpose-group: HBM->HBM bf16 cast (4 SWDGE
            # queues), then transpose-DMAs (sync ring), then natural bf16
            # loads + u tiles ----
            u_alls = {}
            mask_ts = {}
            mt = {}
            for g in range(ngrp):
                gsl = slice(g * GSZ, (g + 1) * GSZ)
                for bt in range(g * bt_per_g, (g + 1) * bt_per_g):
                    bsl = slice(bt * P, (bt + 1) * P)
                    if bt == 0:
                        # split the first cast across the SWDGE queues so
                        # group 0's transposes unblock early
                        for j in range(N):
                            nc.gpsimd.dma_start(out=mbf[bsl, j], in_=mem[bsl, j])
                    else:
                        nc.gpsimd.dma_start(out=mbf[bsl], in_=mem[bsl])
                for j in range(N):
                    # one xbar transpose per head: [GSZ, 512] -> [128, DC, GSZ]
                    # (out dims beyond 2 extend the partition dim logically)
                    t = mtp.tile([P, DC, GSZ], BF16, tag=f"mt{j}")
                    nc.sync.dma_start(
                        out=t[:], in_=mbf[gsl, j, :], transpose=True
                    )
                    for dc in range(DC):
                        mt[g, j, dc] = t[:, dc, :]
                for bt in range(g * bt_per_g, (g + 1) * bt_per_g):
                    bsl = slice(bt * P, (bt + 1) * P)
                    mask_t = maskp.tile([P, N], F32, tag="mask")
                    nc.gpsimd.dma_start(out=mask_t[:], in_=msk[bsl, :, 0])
                    mask_ts[bt] = mask_t
                    m_all = work.tile([P, N * D], BF16, tag="mnat")
                    nc.gpsimd.dma_start(
                        out=m_all.rearrange("p (n d) -> p n d", n=N), in_=mbf[bsl]
                    )
                    # u_j = mask_j * m_j (bf16, 4x tensor_scalar)
                    u_all = upool.tile([P, N * D], BF16, tag="u")
                    for j in range(N):
                        nc.vector.tensor_scalar_mul(
                            out=u_all[:, j * D : (j + 1) * D],
                            in0=m_all[:, j * D : (j + 1) * D],
                            scalar1=mask_t[:, j : j + 1],
                        )
                    u_alls[bt] = u_all

            for bt in range(nbt):
                bsl = slice(bt * P, (bt + 1) * P)
                g, lb = bt // bt_per_g, (bt % bt_per_g) * P
                mask_t = mask_ts[bt]
                u_all = u_alls[bt]

                # ---- matmuls ----
                a_all = work.tile([P, N * D], BF16, tag="a")
                c_all = work.tile([P, N * D], BF16, tag="c")
                for i in range(N):
                    a_ps = psp.tile([P, D], F32, tag="aps")
                    for dc in range(DC):
                        nc.tensor.matmul(
                            out=a_ps[:],
                            lhsT=mt[g, i, dc][:, lb : lb + P],
                            rhs=wt[0, dc][:],
                            start=(dc == 0),
                            stop=False,
                        )
                    nc.tensor.matmul(
                        out=a_ps[:],
                        lhsT=ones_t[:],
                        rhs=bias_t[:],
                        start=False,
                        stop=True,
                    )
                    nc.scalar.copy(out=a_all[:, i * D : (i + 1) * D], in_=a_ps[:])
                for j in range(N):
                    c_ps = psp.tile([P, D], F32, tag="cps")
                    for dc in range(DC):
                        nc.tensor.matmul(
                            out=c_ps[:],
                            lhsT=mt[g, j, dc][:, lb : lb + P],
                            rhs=wt[1, dc][:],
                            start=(dc == 0),
                            stop=(dc == DC - 1),
                        )
                    # c_j = mask_j * (m_j @ W2): scale in the PSUM->SBUF copy
                    nc.scalar.mul(
                        out=c_all[:, j * D : (j + 1) * D],
                        in_=c_ps[:],
                        mul=mask_t[:, j : j + 1],
                    )

                # ---- pairwise sigmoid gating, two i per instruction ----
                for i0 in range(0, N, 2):
                    # t[(i,j)] = a_i + c_j for i in {i0, i0+1}, all j
                    a_b = (
                        a_all[:, i0 * D : (i0 + 2) * D]
                        .rearrange("p (i f) -> p i f", i=2)
                        .rearrange("p i (j f) -> p i j f", j=1)
                        .broadcast_to([P, 2, N, D])
                    )
                    c_b = (
                        c_all.rearrange("p (i f) -> p i f", i=1)
                        .broadcast_to([P, 2, N * D])
                        .rearrange("p i (j f) -> p i j f", j=N)
                    )
                    t_all = pairp.tile([P, 2 * N * D], BF16, tag="t")
                    nc.vector.tensor_tensor(
                        out=t_all.rearrange("p (i j f) -> p i j f", i=2, j=N),
                        in0=a_b,
                        in1=c_b,
                        op=_ADD,
                    )
                    g_all = pairp.tile([P, 2 * N * D], BF16, tag="g")
                    nc.scalar.activation(out=g_all[:], in_=t_all[:], func=_SIGMOID)
                    u_b = (
                        u_all.rearrange("p (i f) -> p i f", i=1)
                        .broadcast_to([P, 2, N * D])
                    )
                    p_all = pairp.tile([P, 2 * N * D], BF16, tag="pp")
                    nc.vector.tensor_tensor(
                        out=p_all.rearrange("p (i f) -> p i f", i=2),
                        in0=g_all.rearrange("p (i f) -> p i f", i=2),
                        in1=u_b,
                        op=_MULT,
                    )
                    for il in range(2):
                        i = i0 + il
                        pv = p_all[:, il * N * D : (il + 1) * N * D].rearrange(
                            "p (j f) -> p j f", j=N
                        )
                        s0, s1, rem = _PAIRS[i]
                        s = accp.tile([P, 2 * D], BF16, tag="s")
                        nc.vector.tensor_tensor(
                            out=s.rearrange("p (j f) -> p j f", j=2),
                            in0=pv[:, s0, :],
                            in1=pv[:, s1, :],
                            op=_ADD,
                        )
                        s2 = accp.tile([P, D], BF16, tag="s2")
                        nc.vector.tensor_add(out=s2[:], in0=s[:, :D], in1=s[:, D:])
                        ctx_t = outp.tile([P, D], F32, tag="ctx")
                        nc.vector.tensor_add(
                            out=ctx_t[:], in0=s2[:], in1=pv[:, rem, :]
                        )
                        nc.scalar.dma_start(out=out[bsl, i, :], in_=ctx_t[:])
    if split_waits:
        _split_excess_waits(nc)
    return nc


def get_nc(bloc=BLOC):
    if bloc not in _nc_cache:
        _nc_cache[bloc] = build(bloc)
    return _nc_cache[bloc]


last_results = None


def kernel(**inputs):
    global last_results
    memory = np.ascontiguousarray(inputs["memory"], dtype=np.float32)
    mask = np.ascontiguousarray(inputs["mask"], dtype=np.float32)
    W = np.ascontiguousarray(inputs["W"], dtype=np.float32)
    b = np.ascontiguousarray(inputs["b"], dtype=np.float32)

    nc = get_nc()
    in_maps = [
        {
            "memory": memory[c * BLOC : (c + 1) * BLOC],
            "mask": mask[c * BLOC : (c + 1) * BLOC],
            "W": W,
            "b": b,
        }
        for c in range(NCORES)
    ]
    res = run_bass_kernel_spmd(nc, in_maps, list(range(NCORES)))
    last_results = res
    out = np.concatenate(
        [res.results[c]["context"] for c in range(NCORES)], axis=0
    )
    return out.astype(np.float32, copy=False)



# revision 2
# speedup vs baseline: 4187.2665x; 4187.2665x over previous
"""Trainium2 Bass kernel for the ContextComputer GNN message-passing module.

Computation (per batch row b):
    W1, W2 = W[:D], W[D:]
    u_j    = memory_j * mask_j                       # [N, D]
    a_i    = memory_i @ W1 + bias                    # [N, D]
    c_j    = mask_j * (memory_j @ W2)                # [N, D]
    ctx_i  = sum_{j != i} sigmoid(a_i + c_j) * u_j

Distribution: the whole batch runs on ONE NeuronCore. In this axon-
tunneled PJRT environment the per-call dispatch overhead scales with the
number of devices in the sharded computation (~1.1 ms per extra device
at steady state) while the device compute for the full B=8192 problem is
only ~1.5-3 ms, so a single-core kernel beats the 8-way data-parallel
split end to end by ~3.5x.

Per-core kernel layout: batch rows on the 128 SBUF partitions (64 row
blocks), feature dim (D=512) on the free axis.
  - memory/W/b are pre-cast to bf16 on the host; the kernel streams
    bf16 and writes a bf16 output that the host upcasts (tolerance is
    rel_err < 2e-2; bf16 compute adds ~4e-3).
  - m^T tiles for the matmuls come from ONE whole-block SBUF->SBUF
    xbar transpose DMA per block ([128, 3072] -> [128, 24, 128]) on the
    sync HWDGE ring - no DRAM bf16 scratch round-trip.
  - u_j = mask_j * m_j on ScalarE (per-partition scale); a'_i = m_i @ W1
    + 1*bias via PSUM accumulation (bias via a rank-1 ones matmul);
    c_j = mask_j * (m_j @ W2) applies the mask as a per-partition scale
    in the ScalarE PSUM->SBUF copy.
  - Pairwise stage: one wide DVE/ACT instruction per two i over all 6 j;
    the off-diagonal j-sum runs on the TensorEngine as 5 identity-lhsT
    matmuls accumulating in PSUM (fp32), freeing the DVE of the adder
    trees; ScalarE copies PSUM -> bf16 out tile, one store DMA per block.

Approx per-block engine budget (errata-adjusted cost model): DVE 19.6us,
ACT 17.1us, PE 18.1us, DMA ~8us -> ~1.3ms/core ideal for 64 blocks.
"""

import numpy as np
import ml_dtypes

import concourse.bass as bass
import concourse.mybir as mybir
import concourse.tile as tile
from concourse.bass_utils import run_bass_kernel_spmd
from concourse.masks import make_identity

B, N, D = 8192, 6, 512
P = 128
DC = D // P  # 4 contraction chunks of 128
NBT = B // P  # 64 row blocks
NCORES = 1
BLOC = B

F32 = mybir.dt.float32
BF16 = mybir.dt.bfloat16

_ADD = mybir.AluOpType.add
_MULT = mybir.AluOpType.mult
_SIGMOID = mybir.ActivationFunctionType.Sigmoid

_nc_cache = {}


def _split_excess_waits(nc, max_waits=1):
    """The pinned walrus build only supports one sync-wait slot per
    instruction; hoist extra Tile-emitted waits onto standalone
    same-engine EventSemaphore instructions (NX dispatcher-level waits,
    so ordering semantics are preserved)."""
    f = nc.m.functions[0]
    for blk in f.blocks:
        new = []
        for ins in blk.instructions:
            si = getattr(ins, "sync_info", None)
            eng = getattr(ins, "engine", None)
            if si is not None and si.on_wait and len(si.on_wait) > max_waits and eng is not None:
                waits = list(si.on_wait)
                extra, keep = waits[:-max_waits], waits[-max_waits:]
                for k, w in enumerate(extra):
                    new.append(
                        mybir.InstEventSemaphore(
                            name=f"{ins.name}_xw{k}",
                            opcode="EventSemaphore",
                            engine=eng,
                            ins=[],
                            outs=[],
                            sync_info=mybir.SyncInfo(on_wait=[w], on_update=[]),
                        )
                    )
                si.on_wait = keep
            new.append(ins)
        blk.instructions[:] = new


def build(bloc=BLOC, split_waits=True):
    nbt = bloc // P
    nc = bass.Bass(num_swdge_queues=4)
    mem = nc.declare_dram_parameter("memory", [bloc, N, D], BF16, isOutput=False)
    msk = nc.declare_dram_parameter("mask", [bloc, N, 1], F32, isOutput=False)
    w_p = nc.declare_dram_parameter("W", [2 * D, D], BF16, isOutput=False)
    b_p = nc.declare_dram_parameter("b", [D], BF16, isOutput=False)
    out = nc.declare_dram_parameter("context", [bloc, N, D], BF16, isOutput=True)

    with tile.TileContext(nc) as tc:
        with (
            tc.tile_pool(name="const", bufs=1) as constp,
            tc.tile_pool(name="mload", bufs=3) as mlp,
            tc.tile_pool(name="mt", bufs=2) as mtp,
            tc.tile_pool(name="ac", bufs=2) as acp,
            tc.tile_pool(name="pair", bufs=2) as pairp,
            tc.tile_pool(name="up", bufs=2) as upool,
            tc.tile_pool(name="maskp", bufs=4) as maskp,
            tc.tile_pool(name="outp", bufs=2) as outp,
            tc.tile_pool(name="psA", bufs=2, space="PSUM") as psA,
            tc.tile_pool(name="psC", bufs=2, space="PSUM") as psC,
            tc.tile_pool(name="psX", bufs=3, space="PSUM") as psX,
        ):
            # ---- constants: W1/W2 tiles, bias, ones row, identity ----
            wt = {}
            for h in range(2):  # 0 -> W1, 1 -> W2
                for dc in range(DC):
                    t = constp.tile([P, D], BF16, tag=f"w{h}{dc}")
                    nc.gpsimd.dma_start(
                        out=t[:], in_=w_p[h * D + dc * P : h * D + (dc + 1) * P, :]
                    )
                    wt[h, dc] = t
            bias_t = constp.tile([1, D], BF16, tag="bias")
            nc.gpsimd.dma_start(out=bias_t[:], in_=b_p[None, :])
            ones_t = constp.tile([1, P], BF16, tag="ones")
            nc.vector.memset(ones_t[:], 1.0)
            ident = constp.tile([P, P], BF16, tag="ident")
            make_identity(nc, ident[:])

            for bt in range(nbt):
                bsl = slice(bt * P, (bt + 1) * P)

                # ---- load block (SWDGE), whole-block SBUF->SBUF transpose ----
                m_all = mlp.tile([P, N * D], BF16, tag="m")
                nc.gpsimd.dma_start(
                    out=m_all.rearrange("p (n d) -> p n d", n=N), in_=mem[bsl]
                )
                mask_t = maskp.tile([P, N], F32, tag="mask")
                nc.gpsimd.dma_start(out=mask_t[:], in_=msk[bsl, :, 0])
                # [128 rows, 3072] -> [128 d, 24, 128 rows]; chunk k = (j, dc)
                mt_all = mtp.tile([P, N * DC, P], BF16, tag="mt")
                nc.sync.dma_start(out=mt_all[:], in_=m_all[:], transpose=True)

                # u_j = mask_j * m_j (ScalarE per-partition scale)
                u_all = upool.tile([P, N * D], BF16, tag="u")
                for j in range(N):
                    nc.scalar.mul(
                        out=u_all[:, j * D : (j + 1) * D],
                        in_=m_all[:, j * D : (j + 1) * D],
                        mul=mask_t[:, j : j + 1],
                    )

                # ---- matmuls ----
                a_all = acp.tile([P, N * D], BF16, tag="a")
                c_all = acp.tile([P, N * D], BF16, tag="c")
                for i in range(N):
                    a_ps = psA.tile([P, D], F32, tag="aps")
                    for dc in range(DC):
                        nc.tensor.matmul(
                            out=a_ps[:],
                            lhsT=mt_all[:, i * DC + dc, :],
                            rhs=wt[0, dc][:],
                            start=(dc == 0),
                            stop=False,
                        )
                    nc.tensor.matmul(
                        out=a_ps[:],
                        lhsT=ones_t[:],
                        rhs=bias_t[:],
                        start=False,
                        stop=True,
                    )
                    nc.scalar.copy(out=a_all[:, i * D : (i + 1) * D], in_=a_ps[:])
                for j in range(N):
                    c_ps = psC.tile([P, D], F32, tag="cps")
                    for dc in range(DC):
                        nc.tensor.matmul(
                            out=c_ps[:],
                            lhsT=mt_all[:, j * DC + dc, :],
                            rhs=wt[1, dc][:],
                            start=(dc == 0),
                            stop=(dc == DC - 1),
                        )
                    # c_j = mask_j * (m_j @ W2): scale in the PSUM->SBUF copy
                    nc.scalar.mul(
                        out=c_all[:, j * D : (j + 1) * D],
                        in_=c_ps[:],
                        mul=mask_t[:, j : j + 1],
                    )

                # ---- pairwise sigmoid gating, two i per instruction ----
                out_all = outp.tile([P, N * D], BF16, tag="o")
                for i0 in range(0, N, 2):
                    # t[(i,j)] = a_i + c_j for i in {i0, i0+1}, all j
                    a_b = (
                        a_all[:, i0 * D : (i0 + 2) * D]
                        .rearrange("p (i f) -> p i f", i=2)
                        .rearrange("p i (j f) -> p i j f", j=1)
                        .broadcast_to([P, 2, N, D])
                    )
                    c_b = (
                        c_all.rearrange("p (i f) -> p i f", i=1)
                        .broadcast_to([P, 2, N * D])
                        .rearrange("p i (j f) -> p i j f", j=N)
                    )
                    t_all = pairp.tile([P, 2 * N * D], BF16, tag="t")
                    nc.vector.tensor_tensor(
                        out=t_all.rearrange("p (i j f) -> p i j f", i=2, j=N),
                        in0=a_b,
                        in1=c_b,
                        op=_ADD,
                    )
                    g_all = pairp.tile([P, 2 * N * D], BF16, tag="g")
                    nc.scalar.activation(out=g_all[:], in_=t_all[:], func=_SIGMOID)
                    u_b = (
                        u_all.rearrange("p (i f) -> p i f", i=1)
                        .broadcast_to([P, 2, N * D])
                    )
                    p_all = pairp.tile([P, 2 * N * D], BF16, tag="pp")
                    nc.vector.tensor_tensor(
                        out=p_all.rearrange("p (i f) -> p i f", i=2),
                        in0=g_all.rearrange("p (i f) -> p i f", i=2),
                        in1=u_b,
                        op=_MULT,
                    )
                    # off-diagonal j-sum on TensorE: 5 identity matmuls into PSUM
                    for il in range(2):
                        i = i0 + il
                        pv = p_all[:, il * N * D : (il + 1) * N * D].rearrange(
                            "p (j f) -> p j f", j=N
                        )
                        x_ps = psX.tile([P, D], F32, tag="xps")
                        js = [j for j in range(N) if j != i]
                        for k, j in enumerate(js):
                            nc.tensor.matmul(
                                out=x_ps[:],
                                lhsT=ident[:],
                                rhs=pv[:, j, :],
                                start=(k == 0),
                                stop=(k == len(js) - 1),
                            )
                        nc.scalar.copy(
                            out=out_all[:, i * D : (i + 1) * D], in_=x_ps[:]
                        )
                nc.gpsimd.dma_start(
                    out=out[bsl], in_=out_all.rearrange("p (n d) -> p n d", n=N)
                )
    if split_waits:
        _split_excess_waits(nc)
    return nc


def get_nc(bloc=BLOC):
    if bloc not in _nc_cache:
        _nc_cache[bloc] = build(bloc)
    return _nc_cache[bloc]


def make_in_maps(inputs):
    """Host-side input staging: pre-cast to the kernel's storage dtypes."""
    memory = np.asarray(inputs["memory"], dtype=np.float32)
    mask = np.ascontiguousarray(np.asarray(inputs["mask"], dtype=np.float32))
    W = np.asarray(inputs["W"], dtype=np.float32)
    b = np.asarray(inputs["b"], dtype=np.float32)
    return [
        {
            "memory": np.ascontiguousarray(memory).astype(ml_dtypes.bfloat16),
            "mask": mask,
            "W": np.ascontiguousarray(W).astype(ml_dtypes.bfloat16),
            "b": np.ascontiguousarray(b).astype(ml_dtypes.bfloat16),
        }
    ]


last_results = None


def kernel(**inputs):
    global last_results
    nc = get_nc()
    in_maps = make_in_maps(inputs)
    res = run_bass_kernel_spmd(nc, in_maps, list(range(NCORES)))
    last_results = res
    out = res.results[0]["context"]
    return np.asarray(out).astype(np.float32)
